# Initial kernel scaffold
#
"""BlockwiseQuantLinear on 8 trn2 NeuronCores.

y = act_quant_dequant(x) @ (fp8_weight * block_scales).T
  x: [8192, 2048] f32, weight: [2048, 2048] fp8_e4m3fn (OCP), w_scale: [16, 16] f32
  out: [8192, 2048] f32

Strategy (data-parallel over tokens; hardcoded shapes):
  - Host: dequantize the static weight to fp16 (exact wrt reference up to fp16
    rounding) and pre-transpose it K-major so [k_inner=128, k_block, n] SBUF
    tiles DMA with 16KB-contiguous rows. Shard x rows 8 ways.
  - Device (per core, M_sh=1024): for each 128-row tile of x: one 1MB load
    (8KB rows); then in two 1024-wide halves: blockwise act quant (amax over
    each (1,128) k-block -> scale; multiply by 224/amax and cast to TRN fp8e4,
    which equals the OCP e4m3fn quantization at half scale -- TRN's max normal
    is 240, so the half grid keeps values <= 224), dequantize to fp16,
    DMA-transpose to [k, m]; then one PSUM-accumulated fp16 GEMM over all 16
    k-blocks per 512-wide n chunk (scales fold fully into the operands, so no
    per-block rescale is needed).
  - DMA rings: x + transposes on sync(SP) with ordering edges (loads of tile
    i+1 after the transpose of tile i, so the scheduler can't starve the PE),
    psum-evict copies + y stores + late weights on scalar(ACT), early weights
    on SWDGE.
  - Gather: concatenate the 8 row shards.
"""

import numpy as np
import ml_dtypes

import concourse.bass as bass
import concourse.mybir as mybir
import concourse.tile as tile
from concourse import bacc
from concourse.bass_utils import run_bass_kernel_spmd
from concourse.masks import make_identity

P = 128
M, K, N = 8192, 2048, 2048
NCORES = 8
M_SH = M // NCORES            # 1024 rows per core
MT = M_SH // P                # 8 m-tiles per core
KB = K // P                   # 16 k blocks
H = 2                         # halves per m-tile (quant/transpose granularity)
KBH = KB // H                 # 8 k blocks per half
KH_W = KBH * P                # 1024
NCH = 4                       # n chunks of 512
NC_W = N // NCH               # 512
EPS = 1e-12

_cache = {}


def _build():
    nc = bacc.Bacc(None, target_bir_lowering=False, num_swdge_queues=4)

    x_in = nc.dram_tensor("x_sh", [M_SH, K], mybir.dt.float32, kind="ExternalInput")
    # [n_chunk, k_inner, k_block, n] -- 16KB contiguous per (c, ki) row
    w_in = nc.dram_tensor(
        "wT", [NCH, P, KB, NC_W], mybir.dt.float16, kind="ExternalInput"
    )
    y_out = nc.dram_tensor("y_sh", [M_SH, N], mybir.dt.float32, kind="ExternalOutput")

    with tile.TileContext(nc) as tc:
        with (
            tc.tile_pool(name="wpool", bufs=1) as wpool,
            tc.tile_pool(name="xpool", bufs=3) as xpool,
            tc.tile_pool(name="qpool", bufs=3) as qpool,
            tc.tile_pool(name="spool", bufs=3) as spool,
            tc.tile_pool(name="ypool", bufs=2) as ypool,
            tc.tile_pool(name="ps", bufs=2, space="PSUM") as ps,
        ):
            # fp16 identity for PE-mode transposes (first pair of tiles only)
            ident = spool.tile([P, P], mybir.dt.float16, name="ident", bufs=1)
            make_identity(nc, ident[:])

            # resident weights: 4 tiles of [128, 16, 512] fp16 (64KB/partition),
            # all on SWDGE, first n-chunk first. The first pair of m-tiles uses
            # PE-mode transposes so no DMA-xbar mode switch blocks the weight
            # stream; psum chains run n-chunk-major per pair, matching the
            # serial weight arrival order.
            wts = []
            for c in range(NCH):
                wt = wpool.tile([P, KB, NC_W], mybir.dt.float16, name=f"w{c}")
                nc.gpsimd.dma_start(wt[:], w_in[c])
                wts.append(wt)

            def quant(xg, h, prev_dve_inst):
                """Emit the act-quant chain for half h of loaded tile xg.
                Returns the dequantized fp16 [P, KH_W] tile and the last DVE
                instruction of the chain."""
                x3 = xg[:, bass.ts(h, KH_W)].rearrange(
                    "p (kb ki) -> p kb ki", kb=KBH
                )
                amax = spool.tile([P, KBH], mybir.dt.float32, name=f"amax{h}", bufs=4)
                rd = nc.vector.tensor_reduce(
                    amax[:], x3, axis=mybir.AxisListType.X,
                    op=mybir.AluOpType.max, apply_absolute_value=True,
                )
                del prev_dve_inst  # DVE-order edges measured slower; not used
                amaxp = spool.tile([P, KBH], mybir.dt.float32, name=f"amaxp{h}", bufs=4)
                nc.vector.tensor_scalar_max(amaxp[:], amax[:], EPS)
                rec = spool.tile([P, KBH], mybir.dt.float32, name=f"rec{h}", bufs=4)
                nc.vector.reciprocal(rec[:], amaxp[:])
                inv2 = spool.tile([P, KBH], mybir.dt.float32, name=f"inv2_{h}", bufs=4)
                nc.vector.tensor_scalar_mul(inv2[:], rec[:], 224.0)
                s2 = spool.tile([P, KBH], mybir.dt.float32, name=f"s2_{h}", bufs=4)
                nc.vector.tensor_scalar_mul(s2[:], amaxp[:], 1.0 / 224.0)

                t8 = qpool.tile([P, KH_W], mybir.dt.float8e4, name=f"t8_{h}", bufs=3)
                t83 = t8[:].rearrange("p (kb ki) -> p kb ki", kb=KBH)
                nc.vector.tensor_tensor(
                    t83, x3, inv2[:, :, None].to_broadcast([P, KBH, P]),
                    mybir.AluOpType.mult,
                )
                xdq = qpool.tile([P, KH_W], mybir.dt.float16, name=f"xdq{h}", bufs=4)
                xdq3 = xdq[:].rearrange("p (kb ki) -> p kb ki", kb=KBH)
                dq = nc.vector.tensor_tensor(
                    xdq3, t83, s2[:, :, None].to_broadcast([P, KBH, P]),
                    mybir.AluOpType.mult,
                )
                return xdq, dq

            PAIR = 2
            prev_xt_inst = None
            prev_dve_inst = None
            for grp in range(MT // PAIR):
                tiles = list(range(grp * PAIR, (grp + 1) * PAIR))
                xdqs = {}
                xTs = {mi: [None] * H for mi in tiles}
                for mi in tiles:
                    xg = xpool.tile([P, K], mybir.dt.float32, name="xg", bufs=3)
                    ld = nc.sync.dma_start(xg[:], x_in[bass.ts(mi, P), :])
                    if prev_xt_inst is not None:
                        tile.add_dep_helper(
                            ld.ins, prev_xt_inst.ins, sync=False,
                            reason="pipeline: next loads after previous transposes",
                        )
                    xdqs[mi] = []
                    for h in range(H):
                        xdq, prev_dve_inst = quant(xg, h, prev_dve_inst)
                        xdqs[mi].append(xdq)
                    if grp == 0:
                        # PE-mode transpose: 8 [128,128] blocks into one fp16
                        # psum bank per half, then a single copy out
                        for h in range(H):
                            tp = ps.tile([P, KH_W], mybir.dt.float16,
                                         name=f"tp{h}", bufs=1)
                            for j in range(KBH):
                                nc.tensor.transpose(
                                    tp[:, bass.ts(j, P)],
                                    xdqs[mi][h][:, bass.ts(j, P)], ident[:],
                                )
                            xT = qpool.tile([P, KBH, P], mybir.dt.float16,
                                            name=f"xT{h}", bufs=PAIR + 4)
                            nc.any.tensor_copy(
                                xT[:].rearrange("p a b -> p (a b)"), tp[:]
                            )
                            xTs[mi][h] = xT

                if grp > 0:
                    # batched xbar transposes (one mode-switch window per pair)
                    for mi in tiles:
                        for h in range(H):
                            xT = qpool.tile([P, KBH, P], mybir.dt.float16,
                                            name=f"xT{h}", bufs=PAIR + 4)
                            prev_xt_inst = nc.sync.dma_start_transpose(
                                xT[:], xdqs[mi][h][:]
                            )
                            xTs[mi][h] = xT

                for c in range(NCH):
                    for mi in tiles:
                        psum = ps.tile([P, NC_W], mybir.dt.float32,
                                       name="psc", bufs=3)
                        for kb in range(KB):
                            h, hk = divmod(kb, KBH)
                            nc.tensor.matmul(
                                psum[:], xTs[mi][h][:, hk, :], wts[c][:, kb, :],
                                start=(kb == 0), stop=(kb == KB - 1),
                            )
                        yc = ypool.tile([P, NC_W], mybir.dt.float32,
                                        name="yc", bufs=4)
                        nc.any.tensor_copy(yc[:], psum[:])
                        nc.scalar.dma_start(
                            y_out[bass.ts(mi, P), bass.ts(c, NC_W)], yc[:]
                        )

    nc.compile()
    return nc


def _prep_weight(weight: np.ndarray, w_scale: np.ndarray) -> np.ndarray:
    w_f32 = weight.astype(np.float32)                     # exact
    ws_full = np.repeat(np.repeat(w_scale.astype(np.float32), P, axis=0), P, axis=1)
    w_deq = (w_f32 * ws_full).astype(np.float16)          # [N, K]
    # w_deq.T[k, n]: k = kb*P + ki, n = c*NC_W + nn -> [c, ki, kb, nn]
    wt = np.ascontiguousarray(
        w_deq.T.reshape(KB, P, NCH, NC_W).transpose(2, 1, 0, 3)
    )
    return wt


def kernel(x: np.ndarray, weight: np.ndarray, w_scale: np.ndarray, _trace: bool = False):
    if "nc" not in _cache:
        _cache["nc"] = _build()
    nc = _cache["nc"]

    weight = np.asarray(weight)
    w_scale = np.asarray(w_scale, dtype=np.float32)
    wt = _prep_weight(weight, w_scale)
    x = np.ascontiguousarray(np.asarray(x), dtype=np.float32)

    in_maps = [
        {"x_sh": x[c * M_SH:(c + 1) * M_SH], "wT": wt}
        for c in range(NCORES)
    ]
    res = run_bass_kernel_spmd(
        nc, in_maps, core_ids=list(range(NCORES)),
        trace=_trace, trace_cores=list(range(NCORES)) if _trace else None,
    )
    y = np.concatenate([res.results[c]["y_sh"] for c in range(NCORES)], axis=0)
    if _trace:
        kernel.last_results = res
    return y



# revision 1
# speedup vs baseline: 1.2227x; 1.2227x over previous
"""BlockwiseQuantLinear on 8 trn2 NeuronCores.

y = act_quant_dequant(x) @ (fp8_weight * block_scales).T
  x: [8192, 2048] f32, weight: [2048, 2048] fp8_e4m3fn (OCP), w_scale: [16, 16] f32
  out: [8192, 2048] f32

Strategy (data-parallel over tokens; hardcoded shapes):
  - Host: dequantize the static weight to fp16 (exact wrt reference up to fp16
    rounding) and pre-transpose it K-major so [k_inner=128, k_block, n] SBUF
    tiles DMA with 16KB-contiguous rows. Shard x rows 8 ways.
  - Device (per core, M_sh=1024): for each 128-row tile of x: one 1MB load
    (8KB rows); then in two 1024-wide halves: blockwise act quant (amax over
    each (1,128) k-block -> scale; multiply by 224/amax and cast to TRN fp8e4,
    which equals the OCP e4m3fn quantization at half scale -- TRN's max normal
    is 240, so the half grid keeps values <= 224), dequantize to fp16,
    DMA-transpose to [k, m]; then one PSUM-accumulated fp16 GEMM over all 16
    k-blocks per 512-wide n chunk (scales fold fully into the operands, so no
    per-block rescale is needed).
  - DMA rings: x + transposes on sync(SP) with ordering edges (loads of tile
    i+1 after the transpose of tile i, so the scheduler can't starve the PE),
    psum-evict copies + y stores + late weights on scalar(ACT), early weights
    on SWDGE.
  - Gather: concatenate the 8 row shards.
"""

import numpy as np
import ml_dtypes

import concourse.bass as bass
import concourse.mybir as mybir
import concourse.tile as tile
from concourse import bacc
from concourse.bass_utils import run_bass_kernel_spmd
from concourse.masks import make_identity

P = 128
M, K, N = 8192, 2048, 2048
NCORES = 8
M_SH = M // NCORES            # 1024 rows per core
MT = M_SH // P                # 8 m-tiles per core
KB = K // P                   # 16 k blocks
H = 2                         # halves per m-tile (quant/transpose granularity)
KBH = KB // H                 # 8 k blocks per half
KH_W = KBH * P                # 1024
NCH = 4                       # n chunks of 512
NC_W = N // NCH               # 512
EPS = 1e-12

_cache = {}


def _build():
    nc = bacc.Bacc(None, target_bir_lowering=False, num_swdge_queues=4)

    x_in = nc.dram_tensor("x_sh", [M_SH, K], mybir.dt.float32, kind="ExternalInput")
    # [n_chunk, k_inner, k_block, n] -- 16KB contiguous per (c, ki) row
    w_in = nc.dram_tensor(
        "wT", [NCH, P, KB, NC_W], mybir.dt.float16, kind="ExternalInput"
    )
    y_out = nc.dram_tensor("y_sh", [M_SH, N], mybir.dt.float32, kind="ExternalOutput")

    with tile.TileContext(nc) as tc:
        with (
            tc.tile_pool(name="wpool", bufs=1) as wpool,
            tc.tile_pool(name="xpool", bufs=3) as xpool,
            tc.tile_pool(name="qpool", bufs=3) as qpool,
            tc.tile_pool(name="spool", bufs=3) as spool,
            tc.tile_pool(name="ypool", bufs=2) as ypool,
            tc.tile_pool(name="ps", bufs=2, space="PSUM") as ps,
        ):
            # fp16 identity for PE-mode transposes (first pair of tiles only)
            ident = spool.tile([P, P], mybir.dt.float16, name="ident", bufs=1)
            make_identity(nc, ident[:])

            # resident weights: 4 tiles of [128, 16, 512] fp16 (64KB/partition),
            # all on SWDGE, first n-chunk first. The first pair of m-tiles uses
            # PE-mode transposes so no DMA-xbar mode switch blocks the weight
            # stream; psum chains run n-chunk-major per pair, matching the
            # serial weight arrival order.
            wts = []
            for c in range(NCH):
                wt = wpool.tile([P, KB, NC_W], mybir.dt.float16, name=f"w{c}")
                nc.gpsimd.dma_start(wt[:], w_in[c])
                wts.append(wt)

            def quant(xg, h, prev_dve_inst):
                """Emit the act-quant chain for half h of loaded tile xg.
                Returns the dequantized fp16 [P, KH_W] tile and the last DVE
                instruction of the chain."""
                x3 = xg[:, bass.ts(h, KH_W)].rearrange(
                    "p (kb ki) -> p kb ki", kb=KBH
                )
                amax = spool.tile([P, KBH], mybir.dt.float32, name=f"amax{h}", bufs=4)
                rd = nc.vector.tensor_reduce(
                    amax[:], x3, axis=mybir.AxisListType.X,
                    op=mybir.AluOpType.max, apply_absolute_value=True,
                )
                del prev_dve_inst  # DVE-order edges measured slower; not used
                amaxp = spool.tile([P, KBH], mybir.dt.float32, name=f"amaxp{h}", bufs=4)
                nc.vector.tensor_scalar_max(amaxp[:], amax[:], EPS)
                rec = spool.tile([P, KBH], mybir.dt.float32, name=f"rec{h}", bufs=4)
                nc.vector.reciprocal(rec[:], amaxp[:])
                inv2 = spool.tile([P, KBH], mybir.dt.float32, name=f"inv2_{h}", bufs=4)
                nc.vector.tensor_scalar_mul(inv2[:], rec[:], 224.0)
                s2 = spool.tile([P, KBH], mybir.dt.float32, name=f"s2_{h}", bufs=4)
                nc.vector.tensor_scalar_mul(s2[:], amaxp[:], 1.0 / 224.0)

                t8 = qpool.tile([P, KH_W], mybir.dt.float8e4, name=f"t8_{h}", bufs=3)
                t83 = t8[:].rearrange("p (kb ki) -> p kb ki", kb=KBH)
                nc.vector.tensor_tensor(
                    t83, x3, inv2[:, :, None].to_broadcast([P, KBH, P]),
                    mybir.AluOpType.mult,
                )
                xdq = qpool.tile([P, KH_W], mybir.dt.float16, name=f"xdq{h}", bufs=4)
                xdq3 = xdq[:].rearrange("p (kb ki) -> p kb ki", kb=KBH)
                dq = nc.vector.tensor_tensor(
                    xdq3, t83, s2[:, :, None].to_broadcast([P, KBH, P]),
                    mybir.AluOpType.mult,
                )
                return xdq, dq

            PAIR = 2
            prev_xt_inst = None
            prev_dve_inst = None
            for grp in range(MT // PAIR):
                tiles = list(range(grp * PAIR, (grp + 1) * PAIR))
                xdqs = {}
                xTs = {mi: [None] * H for mi in tiles}
                for mi in tiles:
                    xg = xpool.tile([P, K], mybir.dt.float32, name="xg", bufs=3)
                    ld = nc.sync.dma_start(xg[:], x_in[bass.ts(mi, P), :])
                    if prev_xt_inst is not None:
                        tile.add_dep_helper(
                            ld.ins, prev_xt_inst.ins, sync=False,
                            reason="pipeline: next loads after previous transposes",
                        )
                    xdqs[mi] = []
                    for h in range(H):
                        xdq, prev_dve_inst = quant(xg, h, prev_dve_inst)
                        xdqs[mi].append(xdq)
                    if grp == 0:
                        # PE-mode transpose: 8 [128,128] blocks into one fp16
                        # psum bank per half, then a single copy out
                        for h in range(H):
                            tp = ps.tile([P, KH_W], mybir.dt.float16,
                                         name=f"tp{h}", bufs=1)
                            for j in range(KBH):
                                nc.tensor.transpose(
                                    tp[:, bass.ts(j, P)],
                                    xdqs[mi][h][:, bass.ts(j, P)], ident[:],
                                )
                            xT = qpool.tile([P, KBH, P], mybir.dt.float16,
                                            name=f"xT{h}", bufs=PAIR + 4)
                            nc.any.tensor_copy(
                                xT[:].rearrange("p a b -> p (a b)"), tp[:]
                            )
                            xTs[mi][h] = xT

                if grp > 0:
                    # batched xbar transposes (one mode-switch window per pair)
                    for mi in tiles:
                        for h in range(H):
                            xT = qpool.tile([P, KBH, P], mybir.dt.float16,
                                            name=f"xT{h}", bufs=PAIR + 4)
                            prev_xt_inst = nc.sync.dma_start_transpose(
                                xT[:], xdqs[mi][h][:]
                            )
                            xTs[mi][h] = xT

                for c in range(NCH):
                    for mi in tiles:
                        psum = ps.tile([P, NC_W], mybir.dt.float32,
                                       name="psc", bufs=3)
                        for kb in range(KB):
                            h, hk = divmod(kb, KBH)
                            nc.tensor.matmul(
                                psum[:], xTs[mi][h][:, hk, :], wts[c][:, kb, :],
                                start=(kb == 0), stop=(kb == KB - 1),
                            )
                        yc = ypool.tile([P, NC_W], mybir.dt.float32,
                                        name="yc", bufs=4)
                        nc.any.tensor_copy(yc[:], psum[:])
                        nc.scalar.dma_start(
                            y_out[bass.ts(mi, P), bass.ts(c, NC_W)], yc[:]
                        )

    nc.compile()
    return nc


def _prep_weight(weight: np.ndarray, w_scale: np.ndarray) -> np.ndarray:
    w_f32 = weight.astype(np.float32)                     # exact
    ws_full = np.repeat(np.repeat(w_scale.astype(np.float32), P, axis=0), P, axis=1)
    w_deq = (w_f32 * ws_full).astype(np.float16)          # [N, K]
    # w_deq.T[k, n]: k = kb*P + ki, n = c*NC_W + nn -> [c, ki, kb, nn]
    wt = np.ascontiguousarray(
        w_deq.T.reshape(KB, P, NCH, NC_W).transpose(2, 1, 0, 3)
    )
    return wt


def kernel(x: np.ndarray, weight: np.ndarray, w_scale: np.ndarray, _trace: bool = False):
    if "nc" not in _cache:
        _cache["nc"] = _build()
    nc = _cache["nc"]

    weight = np.asarray(weight)
    w_scale = np.asarray(w_scale, dtype=np.float32)
    wt = _prep_weight(weight, w_scale)
    x = np.ascontiguousarray(np.asarray(x), dtype=np.float32)

    in_maps = [
        {"x_sh": x[c * M_SH:(c + 1) * M_SH], "wT": wt}
        for c in range(NCORES)
    ]
    res = run_bass_kernel_spmd(
        nc, in_maps, core_ids=list(range(NCORES)),
        trace=_trace, trace_cores=list(range(NCORES)) if _trace else None,
    )
    y = np.concatenate([res.results[c]["y_sh"] for c in range(NCORES)], axis=0)
    if _trace:
        kernel.last_results = res
    return y

